# revision 37
# baseline (speedup 1.0000x reference)
"""Causal self-attention (B=8, T=1024, C=2048, H=16) on 8 TRN2 NeuronCores.

Strategy: data-parallel over batch — core i computes the full attention block
for batch element i (weights replicated, no collectives).

Key optimizations vs the original baseline (667 us -> ~599 us):
  - all weights + x cast to bf16 on the HOST (no on-device cast traffic,
    half the HBM bytes); softmax scale folded into w_q/b_q host-side
  - x PE-transposed in bf16 (1 cycle/row instead of 2 for f32)
  - v computed in natural [T, C] layout directly (xT-stationary matmuls
    against w_v moving) — kills 128 PE transposes + 128 DVE copies
  - per-head pipelining: each head's attention (softmax on ACT/DVE) is
    emitted interleaved with the NEXT head's q/k projection matmuls, so
    the whole middle of the kernel stays PE-bound
  - fine-causal S/exp: S matmuls and exp cover only live columns
    [kt*128, T); permanently-masked eS columns are zeroed once
  - softmax denominators via bf16 tree-adds on the DVE + one ones-matmul
    per 512 queries (was 12 ones-matmuls per head on the PE)
  - w_proj bias applied by the DVE during the PSUM->SBUF copy; last
    projection column runs t-major on preloaded weights so the final
    stores drain early

Per-core pipeline (Tile framework, all matmuls bf16 on the PE):
  A) x [T,C] bf16 -> PE-transpose -> xT; w_v preloads behind it
  B1) v = x @ Wv + bv in natural layout (8 PSUM banks, one per t-tile)
  B2+C) per head: k/q chunk matmuls (W-stationary, xT moving, bias on ACT)
     interleaved with the previous head's S^T = kT^T qT, exp on ACT,
     diagonal-triangle mask, denominator tree, PV accumulation, and the
     1/denom multiply -> attnT bf16
  D) y = attnT-stationary @ w_proj (moving, bf16) + bias, output f32.
"""

import sys

if "/opt/trn_rl_repo" not in sys.path:
    sys.path.insert(0, "/opt/trn_rl_repo")

import numpy as np
import ml_dtypes

import concourse.bass as bass
import concourse.mybir as mybir
import concourse.tile as tile
from concourse import bacc
from concourse.bass_utils import run_bass_kernel_spmd

B, T, C = 8, 1024, 2048
H, HD = 16, 128
N_CORES = 8
P = 128            # partition dim
TQ = 512           # moving-operand tile (q positions per matmul)
KK = C // P        # 16 contraction tiles over C
TT = T // P        # 8 tiles over T
NQ = T // TQ       # 2 q-tiles
NCT = C // TQ      # 4 column tiles over C
SCALE = 1.0 / float(np.sqrt(HD))

f32 = mybir.dt.float32
bf16 = mybir.dt.bfloat16
AFT = mybir.ActivationFunctionType

_NC_CACHE = None


def build_nc():
    nc = bacc.Bacc("TRN2", target_bir_lowering=False, debug=False,
                   num_devices=N_CORES)

    x = nc.declare_dram_parameter("x_bf", [T, C], bf16, isOutput=False)
    # q/k weight chunks, partition-major: wqk[p, m, kk, n] =
    # w_attn[kk*128+p, m*128+n] for m < 32 (q columns pre-scaled)
    wqk = nc.declare_dram_parameter("wqk_pm", [P, 2 * KK, KK, P], bf16,
                                    isOutput=False)
    bqk = nc.declare_dram_parameter("bqk_pm", [P, 2 * KK], f32, isOutput=False)
    wv = nc.declare_dram_parameter("wv_nat", [C, C], bf16, isOutput=False)
    bv = nc.declare_dram_parameter("bv_bc", [P, C], bf16, isOutput=False)
    wp = nc.declare_dram_parameter("wp_nat", [C, C], bf16, isOutput=False)
    bp = nc.declare_dram_parameter("bp_bc", [P, C], bf16, isOutput=False)
    masks = nc.declare_dram_parameter("masks", [P, 4 * TQ], bf16, isOutput=False)
    ident_b = nc.declare_dram_parameter("ident_b", [P, P], bf16, isOutput=False)
    ones_b = nc.declare_dram_parameter("ones_b", [P, P], bf16, isOutput=False)
    y = nc.declare_dram_parameter("y", [T, C], f32, isOutput=True)

    with tile.TileContext(nc) as tc:
        with tc.tile_pool(name="consts", bufs=1) as consts, \
             tc.tile_pool(name="resid", bufs=1) as resid:

            # ---- constants ----
            # identity first (first transposes need it); bulky consts go on
            # the gpsimd queue so they don't delay the x tiles
            identb_sb = consts.tile([P, P], bf16, tag="identb", name="identb")
            nc.sync.dma_start(out=identb_sb, in_=ident_b[:])
            ones_sb = consts.tile([P, P], bf16, tag="ones", name="ones")
            masks_sb = consts.tile([P, 4 * TQ], bf16, tag="masks", name="masks")
            bqk_sb = consts.tile([P, 2 * KK], f32, tag="bqk", name="bqk")
            bv_sb = consts.tile([P, C], bf16, tag="bv", name="bv")
            bp_sb = consts.tile([P, C], bf16, tag="bp", name="bp")

            # ---- persistent intermediates (bf16) ----
            v = [resid.tile([P, C], bf16, tag=f"v{i}", name=f"v{i}") for i in range(TT)]
            attnT = [resid.tile([P, T], bf16, tag=f"attnT{i}", name=f"attnT{i}")
                     for i in range(H)]

            with tc.tile_pool(name="xT", bufs=1) as xTp:
                xT = [xTp.tile([P, T], bf16, tag=f"xT{i}", name=f"xT{i}") for i in range(KK)]

                # ---- Phase A: load x (bf16), PE-transpose into xT; the
                # full w_v streams in behind the x tiles so phase B1 never
                # waits on DMA ----
                with tc.tile_pool(name="wvp", bufs=1) as wvp:
                    wv_sb = [wvp.tile([P, C], bf16, tag=f"wv{kk}",
                                      name=f"wv{kk}") for kk in range(KK)]
                    # w_v streams on the scalar queue while x owns sync
                    for kk in range(KK):
                        nc.scalar.dma_start(out=wv_sb[kk],
                                            in_=wv[kk * P:(kk + 1) * P, :])
                    with tc.tile_pool(name="ldx", bufs=2) as ldx, \
                         tc.tile_pool(name="psA", bufs=3, space=bass.MemorySpace.PSUM) as psA:
                        # 4 double-tile x DMAs on two queues amortize the
                        # ~2us fixed per-DMA latency that starved the
                        # transposes when 8 single-tile DMAs shared one queue
                        x_r = x[:].rearrange("(g t p) c -> g p t c",
                                             g=4, p=P)
                        for g in range(4):
                            x_sb = ldx.tile([P, 2, C], bf16, tag="x_sb",
                                            name="x_sb")
                            eng = nc.sync if g % 2 == 0 else nc.gpsimd
                            eng.dma_start(out=x_sb, in_=x_r[g])
                            for th in range(2):
                                t = 2 * g + th
                                for c in range(KK):
                                    pt = psA.tile([P, P], bf16, tag="pst",
                                                  name="pst")
                                    nc.tensor.transpose(
                                        pt,
                                        x_sb[:, th, c * P:(c + 1) * P],
                                        identb_sb)
                                    nc.vector.tensor_copy(
                                        xT[c][:, t * P:(t + 1) * P], pt)
                        # deferred consts (gpsimd queue, behind the x tiles)
                        nc.gpsimd.dma_start(out=ones_sb, in_=ones_b[:])
                        nc.gpsimd.dma_start(out=masks_sb, in_=masks[:])
                        nc.gpsimd.dma_start(out=bqk_sb, in_=bqk[:])
                        nc.gpsimd.dma_start(out=bv_sb, in_=bv[:])
                        nc.gpsimd.dma_start(out=bp_sb, in_=bp[:])

                    # ---- Phase B1: v = x @ Wv + bv, natural layout ----
                    with tc.tile_pool(name="psV", bufs=8, space=bass.MemorySpace.PSUM) as psVp:
                        for ct in range(NCT):
                            psV = [psVp.tile([P, TQ], f32, tag="psV", name="psV")
                                   for _ in range(TT)]
                            for kk in range(KK):
                                wvt = wv_sb[kk][:, ct * TQ:(ct + 1) * TQ]
                                for t in range(TT):
                                    nc.tensor.matmul(
                                        psV[t], xT[kk][:, t * P:(t + 1) * P], wvt,
                                        start=(kk == 0), stop=(kk == KK - 1))
                            for t in range(TT):
                                nc.vector.tensor_add(
                                    v[t][:, ct * TQ:(ct + 1) * TQ], psV[t],
                                    bv_sb[:, ct * TQ:(ct + 1) * TQ])

                # ---- Merged phase B2+C: per head, the q/k projection
                # chunks (pure PE work) are interleaved with the PREVIOUS
                # head's attention so the softmax's ACT/DVE work and its
                # cross-engine latency hide completely under the projection
                # matmuls.
                #
                # Attention is fine-causal: S matmuls and exp cover only live
                # columns [kt*128, T); the permanently-masked columns of each
                # eS tile are zeroed ONCE and never written again, so the
                # denominator tree-adds can read full 512-wide slices. The
                # diagonal 128x128 triangle is zeroed by a 0/1 mask multiply.
                # PSUM: psB 3 + psS 3 + psO 2 = 8 banks; a matmul with
                # start=True clears its whole bank, so every accumulation
                # group owns a full bank.
                with tc.tile_pool(name="qkp", bufs=2) as qkp, \
                     tc.tile_pool(name="eSp", bufs=1) as eSp, \
                     tc.tile_pool(name="dsc", bufs=2) as dsc, \
                     tc.tile_pool(name="ctmp", bufs=2) as ctmp, \
                     tc.tile_pool(name="wst", bufs=8) as wst, \
                     tc.tile_pool(name="psB", bufs=3, space=bass.MemorySpace.PSUM) as psB, \
                     tc.tile_pool(name="psS", bufs=3, space=bass.MemorySpace.PSUM) as psS, \
                     tc.tile_pool(name="psO", bufs=2, space=bass.MemorySpace.PSUM) as psO:
                    eSab = [[eSp.tile([P, T], bf16, tag=f"eS{s}_{kt}",
                                      name=f"eS{s}_{kt}")
                             for kt in range(TT)] for s in range(2)]
                    for s in range(2):
                        for kt in range(1, TT):
                            nc.vector.memset(eSab[s][kt][:, 0:kt * P], 0)

                    tri = masks_sb[:, 0:P]  # [128,128] q>=k triangle
                    st = [dict() for _ in range(H)]

                    def emit_S(ph, kt):
                        pool, ptag = psS, "psS"
                        q0 = kt * P
                        eS = eSab[ph % 2]
                        kblk = st[ph]["kT"][:, kt * P:(kt + 1) * P]
                        qTt = st[ph]["qT"]
                        if kt < 4:
                            pa = pool.tile([P, TQ], f32, tag=ptag, name="pssa")
                            nc.tensor.matmul(pa[:, q0:TQ], kblk,
                                             qTt[:, q0:TQ],
                                             start=True, stop=True)
                            pb = pool.tile([P, TQ], f32, tag=ptag, name="pssb")
                            nc.tensor.matmul(pb, kblk, qTt[:, TQ:T],
                                             start=True, stop=True)
                            nc.scalar.activation(out=eS[kt][:, q0:TQ],
                                                 in_=pa[:, q0:TQ],
                                                 func=AFT.Exp)
                            nc.scalar.activation(out=eS[kt][:, TQ:T], in_=pb,
                                                 func=AFT.Exp)
                        else:
                            pb = pool.tile([P, TQ], f32, tag=ptag, name="pssb")
                            nc.tensor.matmul(pb[:, q0 - TQ:TQ], kblk,
                                             qTt[:, q0:T],
                                             start=True, stop=True)
                            nc.scalar.activation(out=eS[kt][:, q0:T],
                                                 in_=pb[:, q0 - TQ:TQ],
                                                 func=AFT.Exp)
                        nc.vector.tensor_mul(eS[kt][:, q0:q0 + P],
                                             eS[kt][:, q0:q0 + P], tri)

                    def emit_tree_qt1(ph):
                        eS = eSab[ph % 2]
                        t1 = dsc.tile([P, TQ], bf16, tag="t1", name="t1")
                        t2 = dsc.tile([P, TQ], bf16, tag="t2", name="t2")
                        t3 = dsc.tile([P, TQ], bf16, tag="t3", name="t3")
                        t4 = dsc.tile([P, TQ], bf16, tag="t4", name="t4")
                        nc.vector.tensor_add(t1, eS[0][:, TQ:T], eS[1][:, TQ:T])
                        nc.vector.tensor_add(t2, eS[2][:, TQ:T], eS[3][:, TQ:T])
                        nc.vector.tensor_add(t3, eS[4][:, TQ:T], eS[5][:, TQ:T])
                        nc.vector.tensor_add(t4, eS[6][:, TQ:T], eS[7][:, TQ:T])
                        nc.vector.tensor_add(t1, t1, t2)
                        nc.vector.tensor_add(t3, t3, t4)
                        nc.vector.tensor_add(t1, t1, t3)
                        st[ph]["t1"] = t1

                    def emit_tree_qt0(ph):
                        eS = eSab[ph % 2]
                        u1 = dsc.tile([P, TQ], bf16, tag="t5", name="u1")
                        u2 = dsc.tile([P, TQ], bf16, tag="t6", name="u2")
                        nc.vector.tensor_add(u1, eS[0][:, 0:TQ], eS[1][:, 0:TQ])
                        nc.vector.tensor_add(u2, eS[2][:, 0:TQ], eS[3][:, 0:TQ])
                        nc.vector.tensor_add(u1, u1, u2)
                        st[ph]["u1"] = u1

                    def emit_denoms(ph):
                        psd1 = psS.tile([P, TQ], f32, tag="psS", name="psd1")
                        nc.tensor.matmul(psd1, ones_sb, st[ph]["t1"],
                                         start=True, stop=True)
                        psd0 = psS.tile([P, TQ], f32, tag="psS", name="psd0")
                        nc.tensor.matmul(psd0, ones_sb, st[ph]["u1"],
                                         start=True, stop=True)
                        st[ph]["psd0"], st[ph]["psd1"] = psd0, psd1

                    def emit_PV(ph, kt):
                        eS = eSab[ph % 2]
                        if kt == 0:
                            st[ph]["pso0"] = psO.tile([P, TQ], f32, tag="psO",
                                                      name="pso0")
                            st[ph]["pso1"] = psO.tile([P, TQ], f32, tag="psO",
                                                      name="pso1")
                        lhsT = v[kt][:, ph * P:(ph + 1) * P]
                        # start=True must span the whole bank (it clears it);
                        # accumulating matmuls shrink to the live columns
                        if kt == 0:
                            nc.tensor.matmul(
                                st[ph]["pso0"], lhsT, eS[0][:, 0:TQ],
                                start=True, stop=False)
                            nc.tensor.matmul(
                                st[ph]["pso1"], lhsT, eS[0][:, TQ:T],
                                start=True, stop=False)
                        else:
                            q0 = kt * P
                            if kt < 4:
                                nc.tensor.matmul(
                                    st[ph]["pso0"][:, q0:TQ], lhsT,
                                    eS[kt][:, q0:TQ],
                                    start=False, stop=(kt == 3))
                                nc.tensor.matmul(
                                    st[ph]["pso1"], lhsT, eS[kt][:, TQ:T],
                                    start=False, stop=False)
                            else:
                                lo = max(q0, TQ)
                                nc.tensor.matmul(
                                    st[ph]["pso1"][:, lo - TQ:TQ], lhsT,
                                    eS[kt][:, lo:T],
                                    start=False, stop=(kt == TT - 1))

                    def emit_div(ph):
                        # ~18-bit reciprocal; denominators in [1, ~2e5]
                        rec1 = ctmp.tile([P, TQ], f32, tag="rec", name="rec1")
                        nc.vector.reciprocal_approx_fast(out=rec1,
                                                         in_=st[ph]["psd1"])
                        nc.vector.tensor_mul(attnT[ph][:, TQ:T],
                                             st[ph]["pso1"], rec1)
                        rec0 = ctmp.tile([P, TQ], f32, tag="rec", name="rec0")
                        nc.vector.reciprocal_approx_fast(out=rec0,
                                                         in_=st[ph]["psd0"])
                        nc.vector.tensor_mul(attnT[ph][:, 0:TQ],
                                             st[ph]["pso0"], rec0)
                        st[ph].clear()

                    for i in range(H):
                        ph = i - 1
                        # --- k chunk of head i ---
                        # wqk DMAs issue from the (idle) sync queue — the
                        # scalar sequencer is busy with exps/identities
                        wtk = wst.tile([P, KK, P], bf16, tag="wt", name="wtk")
                        nc.sync.dma_start(out=wtk[:, 0:KK // 2, :],
                                          in_=wqk[:, KK + i, 0:KK // 2, :])
                        nc.sync.dma_start(out=wtk[:, KK // 2:KK, :],
                                          in_=wqk[:, KK + i, KK // 2:KK, :])
                        kTt = qkp.tile([P, T], bf16, tag="kT", name="kTt")
                        psk = [psB.tile([P, TQ], f32, tag="psB", name="psB")
                               for _ in range(NQ)]
                        for kk in range(KK):
                            for qt in range(NQ):
                                nc.tensor.matmul(
                                    psk[qt], wtk[:, kk, :],
                                    xT[kk][:, qt * TQ:(qt + 1) * TQ],
                                    start=(kk == 0), stop=(kk == KK - 1))
                            if ph >= 0 and kk < TT:
                                emit_S(ph, kk)
                        for qt in range(NQ):
                            nc.scalar.activation(
                                out=kTt[:, qt * TQ:(qt + 1) * TQ], in_=psk[qt],
                                func=AFT.Identity,
                                bias=bqk_sb[:, KK + i:KK + i + 1])
                        # --- q chunk of head i ---
                        wtq = wst.tile([P, KK, P], bf16, tag="wt", name="wtq")
                        nc.gpsimd.dma_start(out=wtq[:, 0:KK // 2, :],
                                            in_=wqk[:, i, 0:KK // 2, :])
                        nc.gpsimd.dma_start(out=wtq[:, KK // 2:KK, :],
                                            in_=wqk[:, i, KK // 2:KK, :])
                        qTt = qkp.tile([P, T], bf16, tag="qT", name="qTt")
                        psq = [psB.tile([P, TQ], f32, tag="psB", name="psB")
                               for _ in range(NQ)]
                        for kk in range(KK):
                            for qt in range(NQ):
                                nc.tensor.matmul(
                                    psq[qt], wtq[:, kk, :],
                                    xT[kk][:, qt * TQ:(qt + 1) * TQ],
                                    start=(kk == 0), stop=(kk == KK - 1))
                            if ph >= 0:
                                if kk == 0:
                                    emit_tree_qt1(ph)
                                elif kk == 2:
                                    emit_tree_qt0(ph)
                                elif kk == 5:
                                    emit_denoms(ph)
                                if kk % 2 == 1:
                                    emit_PV(ph, kk // 2)
                        for qt in range(NQ):
                            nc.scalar.activation(
                                out=qTt[:, qt * TQ:(qt + 1) * TQ], in_=psq[qt],
                                func=AFT.Identity, bias=bqk_sb[:, i:i + 1])
                        if ph >= 0:
                            emit_div(ph)
                        st[i]["kT"], st[i]["qT"] = kTt, qTt
                    # --- drain: attention of the last head ---
                    ph = H - 1
                    for kt in range(TT):
                        emit_S(ph, kt)
                    emit_tree_qt1(ph)
                    emit_tree_qt0(ph)
                    emit_denoms(ph)
                    for kt in range(TT):
                        emit_PV(ph, kt)
                    emit_div(ph)

            # ---- Phase D: output projection ----
            # The last column tile runs t-major against preloaded weights so
            # the final bias-adds + stores spread across its whole span
            # instead of bunching at the end of the kernel.
            with tc.tile_pool(name="wpp", bufs=4) as wpp, \
                 tc.tile_pool(name="wpl", bufs=1) as wpl, \
                 tc.tile_pool(name="ybuf", bufs=4) as ybuf:
                  with tc.tile_pool(name="psY", bufs=8, space=bass.MemorySpace.PSUM) as psYp:
                      lct = NCT - 1
                      wpt_l = [wpl.tile([P, TQ], bf16, tag=f"wpl{kk}",
                                        name=f"wpl{kk}") for kk in range(KK)]
                      for kk in range(KK):
                          nc.scalar.dma_start(
                              out=wpt_l[kk],
                              in_=wp[kk * P:(kk + 1) * P,
                                     lct * TQ:(lct + 1) * TQ])
                      for ct in range(NCT - 1):
                          psY = [psYp.tile([P, TQ], f32, tag="psY", name="psY")
                                 for _ in range(TT)]
                          for kk in range(KK):
                              wpt = wpp.tile([P, TQ], bf16, tag="wpt", name="wpt")
                              nc.sync.dma_start(
                                  out=wpt,
                                  in_=wp[kk * P:(kk + 1) * P,
                                         ct * TQ:(ct + 1) * TQ])
                              for t in range(TT):
                                  nc.tensor.matmul(
                                      psY[t], attnT[kk][:, t * P:(t + 1) * P],
                                      wpt, start=(kk == 0), stop=(kk == KK - 1))
                          for t in range(TT):
                              y_sb = ybuf.tile([P, TQ], f32, tag="y_sb", name="y_sb")
                              nc.vector.tensor_add(
                                  y_sb, psY[t], bp_sb[:, ct * TQ:(ct + 1) * TQ])
                              deng = nc.sync if t % 2 == 0 else nc.scalar
                              deng.dma_start(
                                  out=y[t * P:(t + 1) * P,
                                        ct * TQ:(ct + 1) * TQ],
                                  in_=y_sb)
                      # last column tile: t-major
                      psYl = [psYp.tile([P, TQ], f32, tag="psY", name="psYl")
                              for _ in range(TT)]
                      for t in range(TT):
                          for kk in range(KK):
                              nc.tensor.matmul(
                                  psYl[t], attnT[kk][:, t * P:(t + 1) * P],
                                  wpt_l[kk], start=(kk == 0),
                                  stop=(kk == KK - 1))
                          y_sb = ybuf.tile([P, TQ], f32, tag="y_sb", name="y_sb")
                          nc.vector.tensor_add(
                              y_sb, psYl[t], bp_sb[:, lct * TQ:(lct + 1) * TQ])
                          deng = nc.sync if t % 2 == 0 else nc.scalar
                          deng.dma_start(
                              out=y[t * P:(t + 1) * P,
                                    lct * TQ:(lct + 1) * TQ],
                              in_=y_sb)

    nc.compile()
    return nc


def _get_nc():
    global _NC_CACHE
    if _NC_CACHE is None:
        _NC_CACHE = build_nc()
    return _NC_CACHE


def make_in_maps(inputs):
    x = np.asarray(inputs["x"], dtype=np.float32)
    w_attn = np.asarray(inputs["w_attn"], dtype=np.float32)
    b_attn = np.asarray(inputs["b_attn"], dtype=np.float32)
    w_proj = np.asarray(inputs["w_proj"], dtype=np.float32)
    b_proj = np.asarray(inputs["b_proj"], dtype=np.float32)

    bf = ml_dtypes.bfloat16

    # q/k weights, scale folded into q: [P, 32, KK, P] partition-major
    wqk_f = w_attn[:, :2 * C].copy()
    wqk_f[:, :C] *= SCALE
    # [c, n] -> [kk, p, m, n'] -> [p, m, kk, n']
    wqk_pm = np.ascontiguousarray(
        wqk_f.reshape(KK, P, 2 * KK, P).transpose(1, 2, 0, 3)).astype(bf)

    bqk_f = b_attn[:2 * C].copy()
    bqk_f[:C] *= SCALE
    bqk_pm = np.ascontiguousarray(bqk_f.reshape(2 * KK, P).T).astype(np.float32)

    wv_nat = np.ascontiguousarray(w_attn[:, 2 * C:]).astype(bf)
    bv_bc = np.ascontiguousarray(
        np.broadcast_to(b_attn[2 * C:], (P, C))).astype(bf)

    wp_nat = np.ascontiguousarray(w_proj).astype(bf)
    bp_bc = np.ascontiguousarray(np.broadcast_to(b_proj, (P, C))).astype(bf)

    kk_i = np.arange(P)[:, None]
    qq_i = np.arange(TQ)[None, :]
    masks = np.concatenate(
        [(qq_i >= kk_i + P * d) for d in range(4)],
        axis=1).astype(bf)
    ident_b = np.eye(P, dtype=bf)
    ones_b = np.ones((P, P), dtype=bf)

    common = dict(wqk_pm=wqk_pm, bqk_pm=bqk_pm, wv_nat=wv_nat, bv_bc=bv_bc,
                  wp_nat=wp_nat, bp_bc=bp_bc, masks=masks, ident_b=ident_b,
                  ones_b=ones_b)
    x_bf = np.ascontiguousarray(x).astype(bf)
    return [dict(x_bf=np.ascontiguousarray(x_bf[i]), **common)
            for i in range(B)]


def run_spmd(inputs, trace=False, **kw):
    nc = _get_nc()
    in_maps = make_in_maps(inputs)
    return run_bass_kernel_spmd(nc, in_maps, list(range(N_CORES)),
                                trace=trace, **kw)


def kernel(**inputs):
    res = run_spmd(inputs, trace=False)
    y = np.stack([np.asarray(res.results[i]["y"]) for i in range(N_CORES)])
    return y.astype(np.float32)


if __name__ == "__main__":
    rng = np.random.default_rng(0)
    demo = {
        "x": rng.standard_normal((B, T, C)).astype(np.float32),
        "w_attn": (rng.standard_normal((C, 3 * C)) * 0.02).astype(np.float32),
        "b_attn": (rng.standard_normal(3 * C) * 0.02).astype(np.float32),
        "w_proj": (rng.standard_normal((C, C)) * 0.02).astype(np.float32),
        "b_proj": (rng.standard_normal(C) * 0.02).astype(np.float32),
    }
    out = kernel(**demo)
    print("out", out.shape, out.dtype, float(np.abs(out).max()))


# revision 38
# speedup vs baseline: 1.0116x; 1.0116x over previous
"""Causal self-attention (B=8, T=1024, C=2048, H=16) on 8 TRN2 NeuronCores.

Strategy: data-parallel over batch — core i computes the full attention block
for batch element i (weights replicated, no collectives).

Key optimizations vs the original baseline (667 us -> ~599 us):
  - all weights + x cast to bf16 on the HOST (no on-device cast traffic,
    half the HBM bytes); softmax scale folded into w_q/b_q host-side
  - x PE-transposed in bf16 (1 cycle/row instead of 2 for f32)
  - v computed in natural [T, C] layout directly (xT-stationary matmuls
    against w_v moving) — kills 128 PE transposes + 128 DVE copies
  - per-head pipelining: each head's attention (softmax on ACT/DVE) is
    emitted interleaved with the NEXT head's q/k projection matmuls, so
    the whole middle of the kernel stays PE-bound
  - fine-causal S/exp: S matmuls and exp cover only live columns
    [kt*128, T); permanently-masked eS columns are zeroed once
  - softmax denominators via bf16 tree-adds on the DVE + one ones-matmul
    per 512 queries (was 12 ones-matmuls per head on the PE)
  - w_proj bias applied by the DVE during the PSUM->SBUF copy; last
    projection column runs t-major on preloaded weights so the final
    stores drain early

Per-core pipeline (Tile framework, all matmuls bf16 on the PE):
  A) x [T,C] bf16 -> PE-transpose -> xT; w_v preloads behind it
  B1) v = x @ Wv + bv in natural layout (8 PSUM banks, one per t-tile)
  B2+C) per head: k/q chunk matmuls (W-stationary, xT moving, bias on ACT)
     interleaved with the previous head's S^T = kT^T qT, exp on ACT,
     diagonal-triangle mask, denominator tree, PV accumulation, and the
     1/denom multiply -> attnT bf16
  D) y = attnT-stationary @ w_proj (moving, bf16) + bias, output f32.
"""

import sys

if "/opt/trn_rl_repo" not in sys.path:
    sys.path.insert(0, "/opt/trn_rl_repo")

import numpy as np
import ml_dtypes

import concourse.bass as bass
import concourse.mybir as mybir
import concourse.tile as tile
from concourse import bacc
from concourse.bass_utils import run_bass_kernel_spmd

B, T, C = 8, 1024, 2048
H, HD = 16, 128
N_CORES = 8
P = 128            # partition dim
TQ = 512           # moving-operand tile (q positions per matmul)
KK = C // P        # 16 contraction tiles over C
TT = T // P        # 8 tiles over T
NQ = T // TQ       # 2 q-tiles
NCT = C // TQ      # 4 column tiles over C
SCALE = 1.0 / float(np.sqrt(HD))

f32 = mybir.dt.float32
bf16 = mybir.dt.bfloat16
AFT = mybir.ActivationFunctionType

_NC_CACHE = None


def build_nc():
    nc = bacc.Bacc("TRN2", target_bir_lowering=False, debug=False,
                   num_devices=N_CORES)

    x = nc.declare_dram_parameter("x_bf", [T, C], bf16, isOutput=False)
    # q/k weight chunks, partition-major: wqk[p, m, kk, n] =
    # w_attn[kk*128+p, m*128+n] for m < 32 (q columns pre-scaled)
    wqk = nc.declare_dram_parameter("wqk_pm", [P, 2 * KK, KK, P], bf16,
                                    isOutput=False)
    bqk = nc.declare_dram_parameter("bqk_pm", [P, 2 * KK], f32, isOutput=False)
    wv = nc.declare_dram_parameter("wv_nat", [C, C], bf16, isOutput=False)
    bv = nc.declare_dram_parameter("bv_bc", [P, C], bf16, isOutput=False)
    wp = nc.declare_dram_parameter("wp_nat", [C, C], bf16, isOutput=False)
    bp = nc.declare_dram_parameter("bp_bc", [P, C], bf16, isOutput=False)
    masks = nc.declare_dram_parameter("masks", [P, 4 * TQ], bf16, isOutput=False)
    ident_b = nc.declare_dram_parameter("ident_b", [P, P], bf16, isOutput=False)
    ones_b = nc.declare_dram_parameter("ones_b", [P, P], bf16, isOutput=False)
    y = nc.declare_dram_parameter("y", [T, C], f32, isOutput=True)

    with tile.TileContext(nc) as tc:
        with tc.tile_pool(name="consts", bufs=1) as consts, \
             tc.tile_pool(name="resid", bufs=1) as resid:

            # ---- constants ----
            # identity first (first transposes need it); bulky consts go on
            # the gpsimd queue so they don't delay the x tiles
            identb_sb = consts.tile([P, P], bf16, tag="identb", name="identb")
            nc.sync.dma_start(out=identb_sb, in_=ident_b[:])
            ones_sb = consts.tile([P, P], bf16, tag="ones", name="ones")
            nc.gpsimd.dma_start(out=ones_sb, in_=ones_b[:])
            masks_sb = consts.tile([P, 4 * TQ], bf16, tag="masks", name="masks")
            nc.gpsimd.dma_start(out=masks_sb, in_=masks[:])
            bqk_sb = consts.tile([P, 2 * KK], f32, tag="bqk", name="bqk")
            nc.gpsimd.dma_start(out=bqk_sb, in_=bqk[:])
            bv_sb = consts.tile([P, C], bf16, tag="bv", name="bv")
            nc.gpsimd.dma_start(out=bv_sb, in_=bv[:])
            bp_sb = consts.tile([P, C], bf16, tag="bp", name="bp")
            nc.gpsimd.dma_start(out=bp_sb, in_=bp[:])

            # ---- persistent intermediates (bf16) ----
            v = [resid.tile([P, C], bf16, tag=f"v{i}", name=f"v{i}") for i in range(TT)]
            attnT = [resid.tile([P, T], bf16, tag=f"attnT{i}", name=f"attnT{i}")
                     for i in range(H)]

            with tc.tile_pool(name="xT", bufs=1) as xTp:
                xT = [xTp.tile([P, T], bf16, tag=f"xT{i}", name=f"xT{i}") for i in range(KK)]

                # ---- Phase A: load x (bf16), PE-transpose into xT; the
                # full w_v streams in behind the x tiles so phase B1 never
                # waits on DMA ----
                with tc.tile_pool(name="wvp", bufs=1) as wvp:
                    wv_sb = [wvp.tile([P, C], bf16, tag=f"wv{kk}",
                                      name=f"wv{kk}") for kk in range(KK)]
                    # w_v streams on the scalar queue while x owns sync
                    for kk in range(KK):
                        nc.scalar.dma_start(out=wv_sb[kk],
                                            in_=wv[kk * P:(kk + 1) * P, :])
                    with tc.tile_pool(name="ldx", bufs=4) as ldx, \
                         tc.tile_pool(name="psA", bufs=3, space=bass.MemorySpace.PSUM) as psA:
                        for t in range(TT):
                            x_sb = ldx.tile([P, C], bf16, tag="x_sb", name="x_sb")
                            nc.sync.dma_start(out=x_sb,
                                              in_=x[t * P:(t + 1) * P, :])
                            for c in range(KK):
                                pt = psA.tile([P, P], bf16, tag="pst", name="pst")
                                nc.tensor.transpose(pt, x_sb[:, c * P:(c + 1) * P],
                                                    identb_sb)
                                nc.vector.tensor_copy(
                                    xT[c][:, t * P:(t + 1) * P], pt)

                    # ---- Phase B1: v = x @ Wv + bv, natural layout ----
                    with tc.tile_pool(name="psV", bufs=8, space=bass.MemorySpace.PSUM) as psVp:
                        for ct in range(NCT):
                            psV = [psVp.tile([P, TQ], f32, tag="psV", name="psV")
                                   for _ in range(TT)]
                            for kk in range(KK):
                                wvt = wv_sb[kk][:, ct * TQ:(ct + 1) * TQ]
                                for t in range(TT):
                                    nc.tensor.matmul(
                                        psV[t], xT[kk][:, t * P:(t + 1) * P], wvt,
                                        start=(kk == 0), stop=(kk == KK - 1))
                            for t in range(TT):
                                nc.vector.tensor_add(
                                    v[t][:, ct * TQ:(ct + 1) * TQ], psV[t],
                                    bv_sb[:, ct * TQ:(ct + 1) * TQ])

                # ---- Merged phase B2+C: per head, the q/k projection
                # chunks (pure PE work) are interleaved with the PREVIOUS
                # head's attention so the softmax's ACT/DVE work and its
                # cross-engine latency hide completely under the projection
                # matmuls.
                #
                # Attention is fine-causal: S matmuls and exp cover only live
                # columns [kt*128, T); the permanently-masked columns of each
                # eS tile are zeroed ONCE and never written again, so the
                # denominator tree-adds can read full 512-wide slices. The
                # diagonal 128x128 triangle is zeroed by a 0/1 mask multiply.
                # PSUM: psB 3 + psS 3 + psO 2 = 8 banks; a matmul with
                # start=True clears its whole bank, so every accumulation
                # group owns a full bank.
                with tc.tile_pool(name="qkp", bufs=2) as qkp, \
                     tc.tile_pool(name="eSp", bufs=1) as eSp, \
                     tc.tile_pool(name="dsc", bufs=2) as dsc, \
                     tc.tile_pool(name="ctmp", bufs=2) as ctmp, \
                     tc.tile_pool(name="wst", bufs=8) as wst, \
                     tc.tile_pool(name="psB", bufs=3, space=bass.MemorySpace.PSUM) as psB, \
                     tc.tile_pool(name="psS", bufs=3, space=bass.MemorySpace.PSUM) as psS, \
                     tc.tile_pool(name="psO", bufs=2, space=bass.MemorySpace.PSUM) as psO:
                    eSab = [[eSp.tile([P, T], bf16, tag=f"eS{s}_{kt}",
                                      name=f"eS{s}_{kt}")
                             for kt in range(TT)] for s in range(2)]
                    for s in range(2):
                        for kt in range(1, TT):
                            nc.vector.memset(eSab[s][kt][:, 0:kt * P], 0)

                    tri = masks_sb[:, 0:P]  # [128,128] q>=k triangle
                    st = [dict() for _ in range(H)]

                    def emit_S(ph, kt):
                        pool, ptag = psS, "psS"
                        q0 = kt * P
                        eS = eSab[ph % 2]
                        kblk = st[ph]["kT"][:, kt * P:(kt + 1) * P]
                        qTt = st[ph]["qT"]
                        if kt < 4:
                            pa = pool.tile([P, TQ], f32, tag=ptag, name="pssa")
                            nc.tensor.matmul(pa[:, q0:TQ], kblk,
                                             qTt[:, q0:TQ],
                                             start=True, stop=True)
                            pb = pool.tile([P, TQ], f32, tag=ptag, name="pssb")
                            nc.tensor.matmul(pb, kblk, qTt[:, TQ:T],
                                             start=True, stop=True)
                            nc.scalar.activation(out=eS[kt][:, q0:TQ],
                                                 in_=pa[:, q0:TQ],
                                                 func=AFT.Exp)
                            nc.scalar.activation(out=eS[kt][:, TQ:T], in_=pb,
                                                 func=AFT.Exp)
                        else:
                            pb = pool.tile([P, TQ], f32, tag=ptag, name="pssb")
                            nc.tensor.matmul(pb[:, q0 - TQ:TQ], kblk,
                                             qTt[:, q0:T],
                                             start=True, stop=True)
                            nc.scalar.activation(out=eS[kt][:, q0:T],
                                                 in_=pb[:, q0 - TQ:TQ],
                                                 func=AFT.Exp)
                        nc.vector.tensor_mul(eS[kt][:, q0:q0 + P],
                                             eS[kt][:, q0:q0 + P], tri)

                    def emit_tree_qt1(ph):
                        eS = eSab[ph % 2]
                        t1 = dsc.tile([P, TQ], bf16, tag="t1", name="t1")
                        t2 = dsc.tile([P, TQ], bf16, tag="t2", name="t2")
                        t3 = dsc.tile([P, TQ], bf16, tag="t3", name="t3")
                        t4 = dsc.tile([P, TQ], bf16, tag="t4", name="t4")
                        nc.vector.tensor_add(t1, eS[0][:, TQ:T], eS[1][:, TQ:T])
                        nc.vector.tensor_add(t2, eS[2][:, TQ:T], eS[3][:, TQ:T])
                        nc.vector.tensor_add(t3, eS[4][:, TQ:T], eS[5][:, TQ:T])
                        nc.vector.tensor_add(t4, eS[6][:, TQ:T], eS[7][:, TQ:T])
                        nc.vector.tensor_add(t1, t1, t2)
                        nc.vector.tensor_add(t3, t3, t4)
                        nc.vector.tensor_add(t1, t1, t3)
                        st[ph]["t1"] = t1

                    def emit_tree_qt0(ph):
                        eS = eSab[ph % 2]
                        u1 = dsc.tile([P, TQ], bf16, tag="t5", name="u1")
                        u2 = dsc.tile([P, TQ], bf16, tag="t6", name="u2")
                        nc.vector.tensor_add(u1, eS[0][:, 0:TQ], eS[1][:, 0:TQ])
                        nc.vector.tensor_add(u2, eS[2][:, 0:TQ], eS[3][:, 0:TQ])
                        nc.vector.tensor_add(u1, u1, u2)
                        st[ph]["u1"] = u1

                    def emit_denoms(ph):
                        psd1 = psS.tile([P, TQ], f32, tag="psS", name="psd1")
                        nc.tensor.matmul(psd1, ones_sb, st[ph]["t1"],
                                         start=True, stop=True)
                        psd0 = psS.tile([P, TQ], f32, tag="psS", name="psd0")
                        nc.tensor.matmul(psd0, ones_sb, st[ph]["u1"],
                                         start=True, stop=True)
                        st[ph]["psd0"], st[ph]["psd1"] = psd0, psd1

                    def emit_PV(ph, kt):
                        eS = eSab[ph % 2]
                        if kt == 0:
                            st[ph]["pso0"] = psO.tile([P, TQ], f32, tag="psO",
                                                      name="pso0")
                            st[ph]["pso1"] = psO.tile([P, TQ], f32, tag="psO",
                                                      name="pso1")
                        lhsT = v[kt][:, ph * P:(ph + 1) * P]
                        # start=True must span the whole bank (it clears it);
                        # accumulating matmuls shrink to the live columns
                        if kt == 0:
                            nc.tensor.matmul(
                                st[ph]["pso0"], lhsT, eS[0][:, 0:TQ],
                                start=True, stop=False)
                            nc.tensor.matmul(
                                st[ph]["pso1"], lhsT, eS[0][:, TQ:T],
                                start=True, stop=False)
                        else:
                            q0 = kt * P
                            if kt < 4:
                                nc.tensor.matmul(
                                    st[ph]["pso0"][:, q0:TQ], lhsT,
                                    eS[kt][:, q0:TQ],
                                    start=False, stop=(kt == 3))
                                nc.tensor.matmul(
                                    st[ph]["pso1"], lhsT, eS[kt][:, TQ:T],
                                    start=False, stop=False)
                            else:
                                lo = max(q0, TQ)
                                nc.tensor.matmul(
                                    st[ph]["pso1"][:, lo - TQ:TQ], lhsT,
                                    eS[kt][:, lo:T],
                                    start=False, stop=(kt == TT - 1))

                    def emit_div(ph):
                        # ~18-bit reciprocal; denominators in [1, ~2e5]
                        rec1 = ctmp.tile([P, TQ], f32, tag="rec", name="rec1")
                        nc.vector.reciprocal_approx_fast(out=rec1,
                                                         in_=st[ph]["psd1"])
                        nc.vector.tensor_mul(attnT[ph][:, TQ:T],
                                             st[ph]["pso1"], rec1)
                        rec0 = ctmp.tile([P, TQ], f32, tag="rec", name="rec0")
                        nc.vector.reciprocal_approx_fast(out=rec0,
                                                         in_=st[ph]["psd0"])
                        nc.vector.tensor_mul(attnT[ph][:, 0:TQ],
                                             st[ph]["pso0"], rec0)
                        st[ph].clear()

                    for i in range(H):
                        ph = i - 1
                        # --- k chunk of head i ---
                        # wqk DMAs issue from the (idle) sync queue — the
                        # scalar sequencer is busy with exps/identities
                        wtk = wst.tile([P, KK, P], bf16, tag="wt", name="wtk")
                        nc.sync.dma_start(out=wtk[:, 0:KK // 2, :],
                                          in_=wqk[:, KK + i, 0:KK // 2, :])
                        nc.sync.dma_start(out=wtk[:, KK // 2:KK, :],
                                          in_=wqk[:, KK + i, KK // 2:KK, :])
                        kTt = qkp.tile([P, T], bf16, tag="kT", name="kTt")
                        psk = [psB.tile([P, TQ], f32, tag="psB", name="psB")
                               for _ in range(NQ)]
                        for kk in range(KK):
                            for qt in range(NQ):
                                nc.tensor.matmul(
                                    psk[qt], wtk[:, kk, :],
                                    xT[kk][:, qt * TQ:(qt + 1) * TQ],
                                    start=(kk == 0), stop=(kk == KK - 1))
                            if ph >= 0 and kk < TT:
                                emit_S(ph, kk)
                        for qt in range(NQ):
                            nc.scalar.activation(
                                out=kTt[:, qt * TQ:(qt + 1) * TQ], in_=psk[qt],
                                func=AFT.Identity,
                                bias=bqk_sb[:, KK + i:KK + i + 1])
                        # --- q chunk of head i ---
                        wtq = wst.tile([P, KK, P], bf16, tag="wt", name="wtq")
                        nc.gpsimd.dma_start(out=wtq[:, 0:KK // 2, :],
                                            in_=wqk[:, i, 0:KK // 2, :])
                        nc.gpsimd.dma_start(out=wtq[:, KK // 2:KK, :],
                                            in_=wqk[:, i, KK // 2:KK, :])
                        qTt = qkp.tile([P, T], bf16, tag="qT", name="qTt")
                        psq = [psB.tile([P, TQ], f32, tag="psB", name="psB")
                               for _ in range(NQ)]
                        for kk in range(KK):
                            for qt in range(NQ):
                                nc.tensor.matmul(
                                    psq[qt], wtq[:, kk, :],
                                    xT[kk][:, qt * TQ:(qt + 1) * TQ],
                                    start=(kk == 0), stop=(kk == KK - 1))
                            if ph >= 0:
                                if kk == 0:
                                    emit_tree_qt1(ph)
                                elif kk == 2:
                                    emit_tree_qt0(ph)
                                elif kk == 5:
                                    emit_denoms(ph)
                                if kk % 2 == 1:
                                    emit_PV(ph, kk // 2)
                        for qt in range(NQ):
                            nc.scalar.activation(
                                out=qTt[:, qt * TQ:(qt + 1) * TQ], in_=psq[qt],
                                func=AFT.Identity, bias=bqk_sb[:, i:i + 1])
                        if ph >= 0:
                            emit_div(ph)
                        st[i]["kT"], st[i]["qT"] = kTt, qTt
                    # --- drain: attention of the last head ---
                    ph = H - 1
                    for kt in range(TT):
                        emit_S(ph, kt)
                    emit_tree_qt1(ph)
                    emit_tree_qt0(ph)
                    emit_denoms(ph)
                    for kt in range(TT):
                        emit_PV(ph, kt)
                    emit_div(ph)

            # ---- Phase D: output projection ----
            # The last column tile runs t-major against preloaded weights so
            # the final bias-adds + stores spread across its whole span
            # instead of bunching at the end of the kernel.
            with tc.tile_pool(name="wpp", bufs=4) as wpp, \
                 tc.tile_pool(name="wpl", bufs=1) as wpl, \
                 tc.tile_pool(name="ybuf", bufs=4) as ybuf:
                  with tc.tile_pool(name="psY", bufs=8, space=bass.MemorySpace.PSUM) as psYp:
                      lct = NCT - 1
                      wpt_l = [wpl.tile([P, TQ], bf16, tag=f"wpl{kk}",
                                        name=f"wpl{kk}") for kk in range(KK)]
                      for kk in range(KK):
                          nc.scalar.dma_start(
                              out=wpt_l[kk],
                              in_=wp[kk * P:(kk + 1) * P,
                                     lct * TQ:(lct + 1) * TQ])
                      for ct in range(NCT - 1):
                          psY = [psYp.tile([P, TQ], f32, tag="psY", name="psY")
                                 for _ in range(TT)]
                          for kk in range(KK):
                              wpt = wpp.tile([P, TQ], bf16, tag="wpt", name="wpt")
                              nc.sync.dma_start(
                                  out=wpt,
                                  in_=wp[kk * P:(kk + 1) * P,
                                         ct * TQ:(ct + 1) * TQ])
                              for t in range(TT):
                                  nc.tensor.matmul(
                                      psY[t], attnT[kk][:, t * P:(t + 1) * P],
                                      wpt, start=(kk == 0), stop=(kk == KK - 1))
                          for t in range(TT):
                              y_sb = ybuf.tile([P, TQ], f32, tag="y_sb", name="y_sb")
                              nc.vector.tensor_add(
                                  y_sb, psY[t], bp_sb[:, ct * TQ:(ct + 1) * TQ])
                              deng = nc.sync if t % 2 == 0 else nc.scalar
                              deng.dma_start(
                                  out=y[t * P:(t + 1) * P,
                                        ct * TQ:(ct + 1) * TQ],
                                  in_=y_sb)
                      # last column tile: t-major
                      psYl = [psYp.tile([P, TQ], f32, tag="psY", name="psYl")
                              for _ in range(TT)]
                      for t in range(TT):
                          for kk in range(KK):
                              nc.tensor.matmul(
                                  psYl[t], attnT[kk][:, t * P:(t + 1) * P],
                                  wpt_l[kk], start=(kk == 0),
                                  stop=(kk == KK - 1))
                          y_sb = ybuf.tile([P, TQ], f32, tag="y_sb", name="y_sb")
                          nc.vector.tensor_add(
                              y_sb, psYl[t], bp_sb[:, lct * TQ:(lct + 1) * TQ])
                          deng = nc.sync if t % 2 == 0 else nc.scalar
                          deng.dma_start(
                              out=y[t * P:(t + 1) * P,
                                    lct * TQ:(lct + 1) * TQ],
                              in_=y_sb)

    nc.compile()
    return nc


def _get_nc():
    global _NC_CACHE
    if _NC_CACHE is None:
        _NC_CACHE = build_nc()
    return _NC_CACHE


def make_in_maps(inputs):
    x = np.asarray(inputs["x"], dtype=np.float32)
    w_attn = np.asarray(inputs["w_attn"], dtype=np.float32)
    b_attn = np.asarray(inputs["b_attn"], dtype=np.float32)
    w_proj = np.asarray(inputs["w_proj"], dtype=np.float32)
    b_proj = np.asarray(inputs["b_proj"], dtype=np.float32)

    bf = ml_dtypes.bfloat16

    # q/k weights, scale folded into q: [P, 32, KK, P] partition-major
    wqk_f = w_attn[:, :2 * C].copy()
    wqk_f[:, :C] *= SCALE
    # [c, n] -> [kk, p, m, n'] -> [p, m, kk, n']
    wqk_pm = np.ascontiguousarray(
        wqk_f.reshape(KK, P, 2 * KK, P).transpose(1, 2, 0, 3)).astype(bf)

    bqk_f = b_attn[:2 * C].copy()
    bqk_f[:C] *= SCALE
    bqk_pm = np.ascontiguousarray(bqk_f.reshape(2 * KK, P).T).astype(np.float32)

    wv_nat = np.ascontiguousarray(w_attn[:, 2 * C:]).astype(bf)
    bv_bc = np.ascontiguousarray(
        np.broadcast_to(b_attn[2 * C:], (P, C))).astype(bf)

    wp_nat = np.ascontiguousarray(w_proj).astype(bf)
    bp_bc = np.ascontiguousarray(np.broadcast_to(b_proj, (P, C))).astype(bf)

    kk_i = np.arange(P)[:, None]
    qq_i = np.arange(TQ)[None, :]
    masks = np.concatenate(
        [(qq_i >= kk_i + P * d) for d in range(4)],
        axis=1).astype(bf)
    ident_b = np.eye(P, dtype=bf)
    ones_b = np.ones((P, P), dtype=bf)

    common = dict(wqk_pm=wqk_pm, bqk_pm=bqk_pm, wv_nat=wv_nat, bv_bc=bv_bc,
                  wp_nat=wp_nat, bp_bc=bp_bc, masks=masks, ident_b=ident_b,
                  ones_b=ones_b)
    x_bf = np.ascontiguousarray(x).astype(bf)
    return [dict(x_bf=np.ascontiguousarray(x_bf[i]), **common)
            for i in range(B)]


def run_spmd(inputs, trace=False, **kw):
    nc = _get_nc()
    in_maps = make_in_maps(inputs)
    return run_bass_kernel_spmd(nc, in_maps, list(range(N_CORES)),
                                trace=trace, **kw)


def kernel(**inputs):
    res = run_spmd(inputs, trace=False)
    y = np.stack([np.asarray(res.results[i]["y"]) for i in range(N_CORES)])
    return y.astype(np.float32)


if __name__ == "__main__":
    rng = np.random.default_rng(0)
    demo = {
        "x": rng.standard_normal((B, T, C)).astype(np.float32),
        "w_attn": (rng.standard_normal((C, 3 * C)) * 0.02).astype(np.float32),
        "b_attn": (rng.standard_normal(3 * C) * 0.02).astype(np.float32),
        "w_proj": (rng.standard_normal((C, C)) * 0.02).astype(np.float32),
        "b_proj": (rng.standard_normal(C) * 0.02).astype(np.float32),
    }
    out = kernel(**demo)
    print("out", out.shape, out.dtype, float(np.abs(out).max()))
